# revision 58
# baseline (speedup 1.0000x reference)
"""Gemma3 sliding-window attention layer on 8 Trainium2 NeuronCores.

Tensor-parallel over query heads: core h computes query head h (kv head
h//2), i.e. column-parallel qkv projection, full per-head attention, and
the row-parallel slice of o_proj; the 8 partial [S, H] outputs are summed
on the host (the all-reduce / unshard step).

Layout trick: hidden_states is transposed on the host once ([H, S]), so
q and k come out of the projection directly in [d, seq] ("transposed")
layout -- which is exactly the layout both operands of the scores matmul
need (contraction over d on partitions) -- and v comes out natural
[seq, d], which is what the attention-value matmul needs.  The softmax
denominator rides along as a ones-column appended to v, and attn^T (what
o_proj needs as stationary operand) is produced by two PE transposes per
row block.  No other transposes exist anywhere.

RMSNorm per-row reciprocal-RMS values are produced broadcast across all
128 partitions by a ones-matrix matmul (lhsT = ones -> every output
partition holds the per-column sum), so normalizing the [d, seq]-layout
q/k needs no cross-partition shuffles.  RoPE runs on raw cos/sin tables
([128, S], the duplicated-freq halves collapse) with the (1+w) RMSNorm
gains applied as per-partition scalars inside fused scalar_tensor_tensor
DVE ops.

The whole kernel is software-pipelined over the four 512-token column
groups: project group g, norm+rope group g, and interleaved between the
projection sub-phases run attention + o_proj + output-DMA for the four
row blocks of group g-1 (the one-group lag guarantees their inputs are
final).  This keeps the PE busy end to end.
"""

import numpy as np
import ml_dtypes

import concourse.bass as bass
import concourse.mybir as mybir
import concourse.tile as tile
from concourse.bass_utils import run_bass_kernel_spmd
from concourse.masks import make_identity

# ---- problem constants (hardcoded; kernel.py must be self-contained) ----
S = 2048          # sequence length
H = 2560          # hidden size
NH = 8            # query heads
NKV = 4           # kv heads
D = 256           # head dim
EPS = 1e-6
SCALING = 256.0 ** -0.5
WINDOW = 1024 - 1  # sliding window - 1

N_CORES = 8
KC = H // 128      # 20 contraction chunks for the projection
RB = S // 128      # 16 row blocks
GW = 512           # sequence-column group width
NG = S // GW       # 4 groups
MASK_VAL = -1e10

F32 = mybir.dt.float32
BF16 = mybir.dt.bfloat16
BLK_WIN = WINDOW // 128 + 1   # 8: c in [r-8, r] can contribute
MULT = mybir.AluOpType.mult


def _bf16(x):
    return np.ascontiguousarray(x.astype(ml_dtypes.bfloat16))


def _chunk_part(x, p=128):
    """[c*p, n] -> [p, c, n] host relayout so it DMAs 1:1 into an SBUF tile."""
    c = x.shape[0] // p
    return np.ascontiguousarray(
        x.reshape(c, p, *x.shape[1:]).transpose(1, 0, *range(2, x.ndim + 1))
    )


def split_multiwaits(nc):
    """This toolchain's codegen allows one sync-wait slot per instruction.

    Tile emits several waits on the first consumer of multi-queue DMAs and
    on kernel-tail drains; hoist all but the last wait onto same-engine
    NoOps inserted immediately before the offending instruction (queue
    order on the engine preserves the semantics exactly).
    """
    k = 0
    for f in nc.m.functions:
        for bb in f.blocks:
            insts = bb.instructions
            if not any(i.sync_info and len(i.sync_info.on_wait) > 1
                       for i in insts):
                continue
            newlist = []
            for inst in insts:
                si = inst.sync_info
                if si is not None and len(si.on_wait) > 1:
                    for w in list(si.on_wait)[:-1]:
                        nop = mybir.InstNoOp(name=f"{inst.name}-ws{k}")
                        k += 1
                        nop.engine = inst.engine
                        nop.sync_info = mybir.SyncInfo(on_wait=[w], on_update=[])
                        newlist.append(nop)
                    inst.sync_info = mybir.SyncInfo(
                        on_wait=[list(si.on_wait)[-1]],
                        on_update=list(si.on_update))
                newlist.append(inst)
            live = bb.instructions
            live.clear()
            live.extend(newlist)
    return nc


def build_nc():
    """One-core SPMD program (all cores run this; data differs per core)."""
    nc = bass.Bass()

    hT_d = nc.declare_dram_parameter("hT", [128, NG, KC, GW], BF16, isOutput=False)
    wq_d = nc.declare_dram_parameter("wq", [128, KC, D], BF16, isOutput=False)
    wk_d = nc.declare_dram_parameter("wk", [128, KC, D], BF16, isOutput=False)
    wv_d = nc.declare_dram_parameter("wv", [128, KC, D], BF16, isOutput=False)
    cos_d = nc.declare_dram_parameter("cosh", [128, S], BF16, isOutput=False)
    sin_d = nc.declare_dram_parameter("sinh", [128, S], BF16, isOutput=False)
    gains_d = nc.declare_dram_parameter("gains", [128, 8], F32, isOutput=False)
    wo_d = nc.declare_dram_parameter("wo", [128, 2, H], BF16, isOutput=False)
    md_d = nc.declare_dram_parameter("mask_diag", [128, 128], F32, isOutput=False)
    mp_d = nc.declare_dram_parameter("mask_part", [128, 128], F32, isOutput=False)
    ones_d = nc.declare_dram_parameter("ones", [128, 128], BF16, isOutput=False)
    out_d = nc.declare_dram_parameter("out", [S, H], BF16, isOutput=True)

    with tile.TileContext(nc) as tc:
        with (
            tc.tile_pool(name="persist", bufs=1) as persist,
            tc.tile_pool(name="ht", bufs=3) as htpool,
            tc.tile_pool(name="g2", bufs=2) as g2,
            tc.tile_pool(name="g3", bufs=3) as g3,
            tc.tile_pool(name="ptp", bufs=16) as ptp,
            tc.tile_pool(name="ppm", bufs=2, space="PSUM") as ppm,
            tc.tile_pool(name="pop", bufs=2, space="PSUM") as pop,
            tc.tile_pool(name="psc", bufs=2, space="PSUM") as psc,
            tc.tile_pool(name="pav", bufs=1, space="PSUM") as pav,
            tc.tile_pool(name="ptr", bufs=1, space="PSUM") as ptr,
        ):
            # ---- persistent tiles ----
            kTf = persist.tile([128, 2, S], BF16, tag="kTf")
            qTf = persist.tile([128, 2, S], BF16, tag="qTf")
            v_aug = persist.tile([128, RB, D + 1], BF16, tag="vaug")
            cos_sb = persist.tile([128, S], BF16, tag="cos")
            sin_sb = persist.tile([128, S], BF16, tag="sin")
            gains = persist.tile([128, 8], F32, tag="gains")
            wq_sb = persist.tile([128, KC, D], BF16, tag="wq")
            wk_sb = persist.tile([128, KC, D], BF16, tag="wk")
            wv_sb = persist.tile([128, KC, D], BF16, tag="wv")
            wo_sb = persist.tile([128, 2, H], BF16, tag="wo")
            mask_diag = persist.tile([128, 128], F32, tag="md")
            mask_part = persist.tile([128, 128], F32, tag="mp")
            ident = persist.tile([128, 128], BF16, tag="ident")
            ones = persist.tile([128, 128], BF16, tag="ones")
            eps_q = persist.tile([128, 1], F32, tag="eps_q")
            eps_k = persist.tile([128, 1], F32, tag="eps_k")

            # ---- the weights needed first, then cheap init ----
            # (DMA queue order matters: wq chunk 0 + hT chunk 0 gate the
            # first matmul, so they go before everything else)
            for dq in range(5):
                nc.sync.dma_start(out=wq_sb[:, dq * 4:(dq + 1) * 4, :],
                                  in_=wq_d[:, dq * 4:(dq + 1) * 4, :])
            make_identity(nc, ident)
            # dummy matmuls fill the initial DMA-wait window so the PE's
            # HAM clock gate is already at 2.4 GHz when real work arrives
            wps = ppm.tile([128, GW], F32, tag="ppm")
            for _ in range(120):
                nc.tensor.matmul(wps[:, 0:128], lhsT=ident, rhs=ident,
                                 start=True, stop=True)
            nc.vector.memset(eps_q, EPS / (SCALING * SCALING))
            nc.vector.memset(eps_k, EPS)
            nc.vector.memset(v_aug[:, :, D:D + 1], 1.0)

            def load_w(w_sb, w_d):
                for dq in range(5):
                    nc.sync.dma_start(out=w_sb[:, dq * 4:(dq + 1) * 4, :],
                                      in_=w_d[:, dq * 4:(dq + 1) * 4, :])

            # ---------------- emission helpers ----------------
            def proj_qk(ht, w_sb, tag, halves=False):
                """q or k projection for one group -> raw [128, 2, GW] bf16.

                halves=True runs two half-width chains so the first one only
                gates on the first 1.25 MB of hT (group-0 startup).
                """
                raw = g2.tile([128, 2, GW], BF16, tag=tag)
                spans = ((0, 256), (256, 512)) if halves else ((0, GW),)
                for dc in range(2):
                    for a, b in spans:
                        ps = ppm.tile([128, GW], F32, tag="ppm")
                        for kc in range(KC):
                            nc.tensor.matmul(
                                ps[:, 0:b - a],
                                lhsT=w_sb[:, kc, dc * 128:(dc + 1) * 128],
                                rhs=ht[:, kc, a:b],
                                start=(kc == 0), stop=(kc == KC - 1),
                            )
                        if dc == 0:
                            nc.vector.tensor_copy(raw[:, dc, a:b],
                                                  ps[:, 0:b - a])
                        else:
                            nc.scalar.copy(raw[:, dc, a:b], ps[:, 0:b - a])
                return raw

            def proj_v(g, ht):
                for rbg in range(GW // 128):
                    rb = g * (GW // 128) + rbg
                    psv = pav.tile([128, D + 1], F32, tag="pav")
                    for kc in range(KC):
                        nc.tensor.matmul(
                            psv[:, 0:D],
                            lhsT=ht[:, kc, rbg * 128:(rbg + 1) * 128],
                            rhs=wv_sb[:, kc, :],
                            start=(kc == 0), stop=(kc == KC - 1),
                        )
                    nc.vector.tensor_copy(v_aug[:, rb, 0:D], psv[:, 0:D])

            def norm_rope(g, rawq, rawk):
                """RMSNorm stats + normalize + rope for group g (both q,k)."""
                sl = slice(g * GW, (g + 1) * GW)
                for ti, (raw, fin) in enumerate(((rawq, qTf), (rawk, kTf))):
                    sq0 = g2.tile([128, GW], BF16, tag="sq0")
                    sq1 = g2.tile([128, GW], BF16, tag="sq1")
                    nc.scalar.square(sq0, raw[:, 0, :])
                    nc.scalar.square(sq1, raw[:, 1, :])
                    pss = psc.tile([128, GW], F32, tag="psc")
                    nc.tensor.matmul(pss, lhsT=ones, rhs=sq0,
                                     start=True, stop=False)
                    nc.tensor.matmul(pss, lhsT=ones, rhs=sq1,
                                     start=False, stop=True)
                    # rstd = (mean_scaled + eps)^-0.5 via Ln -> Exp(-0.5 x)
                    # (ACT Rsqrt is unavailable; this pair is ~2e-5 accurate
                    # and keeps the reciprocal off the vector engine)
                    lnt = g2.tile([128, GW], F32, tag="lnt")
                    rstd = g2.tile([128, GW], F32, tag="rstd")
                    s2 = SCALING * SCALING if ti == 0 else 1.0
                    nc.scalar.activation(
                        out=lnt, in_=pss,
                        func=mybir.ActivationFunctionType.Ln,
                        scale=1.0 / (D * s2),
                        bias=(eps_q if ti == 0 else eps_k),
                    )
                    nc.scalar.activation(
                        out=rstd, in_=lnt,
                        func=mybir.ActivationFunctionType.Exp,
                        scale=-0.5,
                    )
                    # normalize in place (bf16 <- bf16 * f32)
                    for dc in range(2):
                        nc.vector.tensor_mul(raw[:, dc, :], raw[:, dc, :], rstd)
                    # rope: fin[dc] = cos*g0*raw[dc] + sin*g1*raw[1-dc]
                    for dc in range(2):
                        gb = ti * 4 + dc * 2
                        t1 = g2.tile([128, GW], BF16, tag="t1")
                        t2 = g2.tile([128, GW], BF16, tag="t2")
                        nc.vector.scalar_tensor_tensor(
                            out=t1, in0=cos_sb[:, sl],
                            scalar=gains[:, gb:gb + 1],
                            in1=raw[:, dc, :], op0=MULT, op1=MULT)
                        nc.vector.scalar_tensor_tensor(
                            out=t2, in0=sin_sb[:, sl],
                            scalar=gains[:, gb + 1:gb + 2],
                            in1=raw[:, 1 - dc, :], op0=MULT, op1=MULT)
                        nc.vector.tensor_add(fin[:, dc, sl], t1, t2)

            def attn_scores(G):
                """Scores + exp for attention group G (query rows 4G..4G+3).

                One matmul per (key block, dc), exactly as wide as the query
                rows whose sliding window contains that key block (1..4 row
                blocks, contiguous) -- no wasted score columns, and one
                LDWEIGHTS serves up to four rows.  Returns {c: (pT, qstart)}.
                """
                r0 = 4 * G
                pts = {}
                for c in range(max(0, r0 - BLK_WIN), r0 + 4):
                    qs = max(c, r0)                 # first row needing c
                    qe = min(c + BLK_WIN, r0 + 3)   # last row needing c
                    w = (qe - qs + 1) * 128
                    psW = psc.tile([128, GW], F32, tag="psc")
                    for dc in range(2):
                        nc.tensor.matmul(
                            psW[:, 0:w],
                            lhsT=kTf[:, dc, c * 128:(c + 1) * 128],
                            rhs=qTf[:, dc, qs * 128:(qe + 1) * 128],
                            start=(dc == 0), stop=(dc == 1),
                        )
                    if c >= qs and c <= qe:         # diagonal block (c == r)
                        j = (c - qs) * 128
                        nc.vector.tensor_add(
                            psW[:, j:j + 128], psW[:, j:j + 128], mask_diag)
                    if c + BLK_WIN <= qe:           # partial-window block
                        j = (c + BLK_WIN - qs) * 128
                        nc.vector.tensor_add(
                            psW[:, j:j + 128], psW[:, j:j + 128], mask_part)
                    pT = ptp.tile([128, GW], BF16, tag="pT")
                    nc.scalar.activation(
                        out=pT[:, 0:w], in_=psW[:, 0:w],
                        func=mybir.ActivationFunctionType.Exp,
                    )
                    pts[c] = (pT, qs)
                return pts

            def attn_row(r, pts, split_out=False, use_ppm=False):
                """attn-value accumulation + o_proj + out DMA for row r.

                use_ppm: after the last projection the ppm pool is idle, so
                late rows can accumulate there and overlap pav-pool rows.
                """
                cmin = max(0, r - BLK_WIN)
                if use_ppm:
                    ps_w = ppm.tile([128, GW], F32, tag="ppm")
                    ps_at = ps_w[:, 0:D + 1]
                else:
                    ps_at = pav.tile([128, D + 1], F32, tag="pav")
                for c in range(cmin, r + 1):
                    pT, qs = pts[c]
                    j = (r - qs) * 128
                    nc.tensor.matmul(
                        ps_at,
                        lhsT=pT[:, j:j + 128],
                        rhs=v_aug[:, c, :],
                        start=(c == cmin), stop=(c == r),
                    )
                rc = g2.tile([128, 1], F32, tag="rc")
                nc.vector.reciprocal(rc, ps_at[:, D:D + 1])
                a_sb = g2.tile([128, D], BF16, tag="asb")
                nc.scalar.mul(a_sb, ps_at[:, 0:D], rc)
                pt = ptr.tile([128, 2 * 128], BF16, tag="ptr")
                for dc in range(2):
                    nc.tensor.transpose(
                        pt[:, dc * 128:(dc + 1) * 128],
                        a_sb[:, dc * 128:(dc + 1) * 128], ident)
                attnT = g3.tile([128, 2 * 128], BF16, tag="attnT")
                nc.vector.tensor_copy(attnT, pt)
                # o_proj for this row block
                o_sb = g3.tile([128, H], BF16, tag="osb")
                for hc in range(H // GW):
                    ps = pop.tile([128, GW], F32, tag="pop")
                    for dc in range(2):
                        nc.tensor.matmul(
                            ps,
                            lhsT=attnT[:, dc * 128:(dc + 1) * 128],
                            rhs=wo_sb[:, dc, hc * GW:(hc + 1) * GW],
                            start=(dc == 0), stop=(dc == 1),
                        )
                    if use_ppm or hc % 2 == 0:
                        # late rows: keep every copy on ACT -- the vector
                        # engine is the bottleneck during the last rope
                        nc.scalar.copy(o_sb[:, hc * GW:(hc + 1) * GW], ps)
                    else:
                        nc.vector.tensor_copy(o_sb[:, hc * GW:(hc + 1) * GW], ps)
                    if split_out:
                        # kernel tail: stream each chunk out as soon as its
                        # copy lands so the final DMA is small
                        nc.sync.dma_start(
                            out=out_d[r * 128:(r + 1) * 128,
                                      hc * GW:(hc + 1) * GW],
                            in_=o_sb[:, hc * GW:(hc + 1) * GW])
                if not split_out:
                    nc.sync.dma_start(out=out_d[r * 128:(r + 1) * 128, :],
                                      in_=o_sb)

            # ---------------- pipelined main loop ----------------
            ht_tiles = {}

            def fetch_ht(g):
                ht = htpool.tile([128, KC, GW], BF16, tag="ht")
                for dq in range(5):
                    nc.sync.dma_start(
                        out=ht[:, dq * 4:(dq + 1) * 4, :],
                        in_=hT_d[:, g, dq * 4:(dq + 1) * 4, :],
                    )
                ht_tiles[g] = ht

            fetch_ht(0)
            for g in range(NG):
                ht = ht_tiles.pop(g)
                rawq = proj_qk(ht, wq_sb, "rawq")
                if g == 0:
                    load_w(wk_sb, wk_d)
                else:
                    if g + 1 < NG:
                        fetch_ht(g + 1)
                    pts = attn_scores(g - 1)
                    attn_row(4 * (g - 1) + 0, pts)
                rawk = proj_qk(ht, wk_sb, "rawk")
                if g == 0:
                    load_w(wv_sb, wv_d)
                    fetch_ht(1)
                else:
                    attn_row(4 * (g - 1) + 1, pts)
                proj_v(g, ht)
                if g == 0:
                    nc.sync.dma_start(out=mask_diag, in_=md_d[:])
                    nc.sync.dma_start(out=mask_part, in_=mp_d[:])
                    nc.sync.dma_start(out=ones, in_=ones_d[:])
                    nc.sync.dma_start(out=gains, in_=gains_d[:])
                    nc.sync.dma_start(out=cos_sb, in_=cos_d[:])
                    nc.sync.dma_start(out=sin_sb, in_=sin_d[:])
                    nc.sync.dma_start(out=wo_sb, in_=wo_d[:])
                else:
                    attn_row(4 * (g - 1) + 2, pts)
                    if g < NG - 1:
                        attn_row(4 * (g - 1) + 3, pts)
                norm_rope(g, rawq, rawk)
                if g == NG - 1:
                    # row 11 lands here so its matmuls cover the rope-3 wait
                    attn_row(4 * (g - 1) + 3, pts, use_ppm=True)
                    pts = attn_scores(NG - 1)
                    attn_row(12, pts)
                    attn_row(13, pts, use_ppm=True)
            attn_row(14, pts, split_out=True)
            attn_row(15, pts, split_out=True, use_ppm=True)

    return nc


def make_in_maps(hidden_states, cos, sin, w_qkv, w_o, q_norm_w, k_norm_w):
    """Host-side sharding / relayout: one input map per core."""
    f32 = np.float32
    hT = _chunk_part(np.ascontiguousarray(hidden_states.T).astype(f32))
    # regroup to [128, 4 seq-groups, KC, 512] so each 512-col group loads
    # with a few large contiguous DMAs
    hT = _bf16(np.ascontiguousarray(
        hT.reshape(128, KC, NG, GW).transpose(0, 2, 1, 3)))

    # cos/sin have the freqs duplicated along the last dim -> keep one half,
    # transposed to [128, S]
    cosh = _bf16(np.ascontiguousarray(np.asarray(cos, dtype=f32).T[:128]))
    sinh = _bf16(np.ascontiguousarray(np.asarray(sin, dtype=f32).T[:128]))

    def gain_cols(w):
        w1 = 1.0 + np.asarray(w, dtype=f32)
        lo, hi = w1[:128], w1[128:]
        # [t1 dc0, t2 dc0, t1 dc1, t2 dc1]
        return [lo, -hi, hi, lo]

    gains = np.stack(gain_cols(q_norm_w) + gain_cols(k_norm_w),
                     axis=1).astype(f32)  # [128, 8]

    jj = np.arange(128)[:, None]  # key index within block (partition)
    ii = np.arange(128)[None, :]  # query index within block (free)
    mask_diag = np.where(jj <= ii, 0.0, MASK_VAL).astype(f32)
    mask_part = np.where(jj >= ii + 1, 0.0, MASK_VAL).astype(f32)
    ones_h = _bf16(np.ones((128, 128), f32))

    in_maps = []
    for h in range(N_CORES):
        g = h // (NH // NKV)
        wq = _bf16(_chunk_part(np.ascontiguousarray(
            w_qkv[:, h * D:(h + 1) * D]).astype(f32)))
        wk = _bf16(_chunk_part(np.ascontiguousarray(
            w_qkv[:, NH * D + g * D: NH * D + (g + 1) * D]).astype(f32)))
        wv = _bf16(_chunk_part(np.ascontiguousarray(
            w_qkv[:, (NH + NKV) * D + g * D: (NH + NKV) * D + (g + 1) * D]
        ).astype(f32)))
        wo = _bf16(_chunk_part(np.ascontiguousarray(
            w_o[h * D:(h + 1) * D, :]).astype(f32)))
        in_maps.append({
            "hT": hT, "wq": wq, "wk": wk, "wv": wv,
            "cosh": cosh, "sinh": sinh, "gains": gains,
            "wo": wo, "mask_diag": mask_diag, "mask_part": mask_part,
            "ones": ones_h,
        })
    return in_maps


_NC_CACHE = None


def _get_nc():
    global _NC_CACHE
    if _NC_CACHE is None:
        _NC_CACHE = split_multiwaits(build_nc())
    return _NC_CACHE


def run(inputs, trace=False, **kw):
    """Returns (full_output, BassKernelResults)."""
    nc = _get_nc()
    in_maps = make_in_maps(**inputs)
    res = run_bass_kernel_spmd(
        nc, in_maps, core_ids=list(range(N_CORES)), trace=trace, **kw
    )
    parts = [res.results[i]["out"].astype(np.float32) for i in range(N_CORES)]
    out = np.sum(np.stack(parts, axis=0), axis=0, dtype=np.float32)
    return out, res


def kernel(**inputs) -> np.ndarray:
    out, _ = run(inputs, trace=False)
    return out


# revision 59
# speedup vs baseline: 1.0098x; 1.0098x over previous
"""Gemma3 sliding-window attention layer on 8 Trainium2 NeuronCores.

Tensor-parallel over query heads: core h computes query head h (kv head
h//2), i.e. column-parallel qkv projection, full per-head attention, and
the row-parallel slice of o_proj; the 8 partial [S, H] outputs are summed
on the host (the all-reduce / unshard step).

Layout trick: hidden_states is transposed on the host once ([H, S]), so
q and k come out of the projection directly in [d, seq] ("transposed")
layout -- which is exactly the layout both operands of the scores matmul
need (contraction over d on partitions) -- and v comes out natural
[seq, d], which is what the attention-value matmul needs.  The softmax
denominator rides along as a ones-column appended to v, and attn^T (what
o_proj needs as stationary operand) is produced by two PE transposes per
row block.  No other transposes exist anywhere.

RMSNorm per-row reciprocal-RMS values are produced broadcast across all
128 partitions by a ones-matrix matmul (lhsT = ones -> every output
partition holds the per-column sum), so normalizing the [d, seq]-layout
q/k needs no cross-partition shuffles.  RoPE runs on raw cos/sin tables
([128, S], the duplicated-freq halves collapse) with the (1+w) RMSNorm
gains applied as per-partition scalars inside fused scalar_tensor_tensor
DVE ops.

The whole kernel is software-pipelined over the four 512-token column
groups: project group g, norm+rope group g, and interleaved between the
projection sub-phases run attention + o_proj + output-DMA for the four
row blocks of group g-1 (the one-group lag guarantees their inputs are
final).  This keeps the PE busy end to end.
"""

import numpy as np
import ml_dtypes

import concourse.bass as bass
import concourse.mybir as mybir
import concourse.tile as tile
from concourse.bass_utils import run_bass_kernel_spmd
from concourse.masks import make_identity

# ---- problem constants (hardcoded; kernel.py must be self-contained) ----
S = 2048          # sequence length
H = 2560          # hidden size
NH = 8            # query heads
NKV = 4           # kv heads
D = 256           # head dim
EPS = 1e-6
SCALING = 256.0 ** -0.5
WINDOW = 1024 - 1  # sliding window - 1

N_CORES = 8
KC = H // 128      # 20 contraction chunks for the projection
RB = S // 128      # 16 row blocks
GW = 512           # sequence-column group width
NG = S // GW       # 4 groups
MASK_VAL = -1e10

F32 = mybir.dt.float32
BF16 = mybir.dt.bfloat16
BLK_WIN = WINDOW // 128 + 1   # 8: c in [r-8, r] can contribute
MULT = mybir.AluOpType.mult


def _bf16(x):
    return np.ascontiguousarray(x.astype(ml_dtypes.bfloat16))


def _chunk_part(x, p=128):
    """[c*p, n] -> [p, c, n] host relayout so it DMAs 1:1 into an SBUF tile."""
    c = x.shape[0] // p
    return np.ascontiguousarray(
        x.reshape(c, p, *x.shape[1:]).transpose(1, 0, *range(2, x.ndim + 1))
    )


def split_multiwaits(nc):
    """This toolchain's codegen allows one sync-wait slot per instruction.

    Tile emits several waits on the first consumer of multi-queue DMAs and
    on kernel-tail drains; hoist all but the last wait onto same-engine
    NoOps inserted immediately before the offending instruction (queue
    order on the engine preserves the semantics exactly).
    """
    k = 0
    for f in nc.m.functions:
        for bb in f.blocks:
            insts = bb.instructions
            if not any(i.sync_info and len(i.sync_info.on_wait) > 1
                       for i in insts):
                continue
            newlist = []
            for inst in insts:
                si = inst.sync_info
                if si is not None and len(si.on_wait) > 1:
                    for w in list(si.on_wait)[:-1]:
                        nop = mybir.InstNoOp(name=f"{inst.name}-ws{k}")
                        k += 1
                        nop.engine = inst.engine
                        nop.sync_info = mybir.SyncInfo(on_wait=[w], on_update=[])
                        newlist.append(nop)
                    inst.sync_info = mybir.SyncInfo(
                        on_wait=[list(si.on_wait)[-1]],
                        on_update=list(si.on_update))
                newlist.append(inst)
            live = bb.instructions
            live.clear()
            live.extend(newlist)
    return nc


def build_nc():
    """One-core SPMD program (all cores run this; data differs per core)."""
    nc = bass.Bass()

    hT_d = nc.declare_dram_parameter("hT", [128, NG, KC, GW], BF16, isOutput=False)
    wq_d = nc.declare_dram_parameter("wq", [128, KC, D], BF16, isOutput=False)
    wk_d = nc.declare_dram_parameter("wk", [128, KC, D], BF16, isOutput=False)
    wv_d = nc.declare_dram_parameter("wv", [128, KC, D], BF16, isOutput=False)
    cos_d = nc.declare_dram_parameter("cosh", [128, S], BF16, isOutput=False)
    sin_d = nc.declare_dram_parameter("sinh", [128, S], BF16, isOutput=False)
    gains_d = nc.declare_dram_parameter("gains", [128, 8], F32, isOutput=False)
    wo_d = nc.declare_dram_parameter("wo", [128, 2, H], BF16, isOutput=False)
    md_d = nc.declare_dram_parameter("mask_diag", [128, 128], F32, isOutput=False)
    mp_d = nc.declare_dram_parameter("mask_part", [128, 128], F32, isOutput=False)
    ones_d = nc.declare_dram_parameter("ones", [128, 128], BF16, isOutput=False)
    out_d = nc.declare_dram_parameter("out", [S, H], BF16, isOutput=True)

    with tile.TileContext(nc) as tc:
        with (
            tc.tile_pool(name="persist", bufs=1) as persist,
            tc.tile_pool(name="ht", bufs=3) as htpool,
            tc.tile_pool(name="g2", bufs=2) as g2,
            tc.tile_pool(name="g3", bufs=3) as g3,
            tc.tile_pool(name="ptp", bufs=16) as ptp,
            tc.tile_pool(name="ppm", bufs=2, space="PSUM") as ppm,
            tc.tile_pool(name="pop", bufs=2, space="PSUM") as pop,
            tc.tile_pool(name="psc", bufs=2, space="PSUM") as psc,
            tc.tile_pool(name="pav", bufs=1, space="PSUM") as pav,
            tc.tile_pool(name="ptr", bufs=1, space="PSUM") as ptr,
        ):
            # ---- persistent tiles ----
            kTf = persist.tile([128, 2, S], BF16, tag="kTf")
            qTf = persist.tile([128, 2, S], BF16, tag="qTf")
            v_aug = persist.tile([128, RB, D + 1], BF16, tag="vaug")
            cos_sb = persist.tile([128, S], BF16, tag="cos")
            sin_sb = persist.tile([128, S], BF16, tag="sin")
            gains = persist.tile([128, 8], F32, tag="gains")
            wq_sb = persist.tile([128, KC, D], BF16, tag="wq")
            wk_sb = persist.tile([128, KC, D], BF16, tag="wk")
            wv_sb = persist.tile([128, KC, D], BF16, tag="wv")
            wo_sb = persist.tile([128, 2, H], BF16, tag="wo")
            mask_diag = persist.tile([128, 128], F32, tag="md")
            mask_part = persist.tile([128, 128], F32, tag="mp")
            ident = persist.tile([128, 128], BF16, tag="ident")
            ones = persist.tile([128, 128], BF16, tag="ones")
            eps_q = persist.tile([128, 1], F32, tag="eps_q")
            eps_k = persist.tile([128, 1], F32, tag="eps_k")

            # ---- the weights needed first, then cheap init ----
            # (DMA queue order matters: wq chunk 0 + hT chunk 0 gate the
            # first matmul, so they go before everything else)
            for dq in range(5):
                nc.sync.dma_start(out=wq_sb[:, dq * 4:(dq + 1) * 4, :],
                                  in_=wq_d[:, dq * 4:(dq + 1) * 4, :])
            make_identity(nc, ident)
            # dummy matmuls fill the initial DMA-wait window so the PE's
            # HAM clock gate is already at 2.4 GHz when real work arrives
            wps = ppm.tile([128, GW], F32, tag="ppm")
            for _ in range(120):
                nc.tensor.matmul(wps[:, 0:128], lhsT=ident, rhs=ident,
                                 start=True, stop=True)
            nc.vector.memset(eps_q, EPS / (SCALING * SCALING))
            nc.vector.memset(eps_k, EPS)
            nc.vector.memset(v_aug[:, :, D:D + 1], 1.0)

            def load_w(w_sb, w_d):
                for dq in range(5):
                    nc.sync.dma_start(out=w_sb[:, dq * 4:(dq + 1) * 4, :],
                                      in_=w_d[:, dq * 4:(dq + 1) * 4, :])

            # ---------------- emission helpers ----------------
            def proj_qk(ht, w_sb, tag, halves=False):
                """q or k projection for one group -> raw [128, 2, GW] bf16.

                halves=True runs two half-width chains so the first one only
                gates on the first 1.25 MB of hT (group-0 startup).
                """
                raw = g2.tile([128, 2, GW], BF16, tag=tag)
                spans = ((0, 256), (256, 512)) if halves else ((0, GW),)
                for dc in range(2):
                    for a, b in spans:
                        ps = ppm.tile([128, GW], F32, tag="ppm")
                        for kc in range(KC):
                            nc.tensor.matmul(
                                ps[:, 0:b - a],
                                lhsT=w_sb[:, kc, dc * 128:(dc + 1) * 128],
                                rhs=ht[:, kc, a:b],
                                start=(kc == 0), stop=(kc == KC - 1),
                            )
                        if dc == 0:
                            nc.vector.tensor_copy(raw[:, dc, a:b],
                                                  ps[:, 0:b - a])
                        else:
                            nc.scalar.copy(raw[:, dc, a:b], ps[:, 0:b - a])
                return raw

            def proj_v(g, ht):
                for rbg in range(GW // 128):
                    rb = g * (GW // 128) + rbg
                    psv = pav.tile([128, D + 1], F32, tag="pav")
                    for kc in range(KC):
                        nc.tensor.matmul(
                            psv[:, 0:D],
                            lhsT=ht[:, kc, rbg * 128:(rbg + 1) * 128],
                            rhs=wv_sb[:, kc, :],
                            start=(kc == 0), stop=(kc == KC - 1),
                        )
                    nc.vector.tensor_copy(v_aug[:, rb, 0:D], psv[:, 0:D])

            def norm_rope(g, rawq, rawk):
                """RMSNorm stats + normalize + rope for group g (both q,k)."""
                sl = slice(g * GW, (g + 1) * GW)
                for ti, (raw, fin) in enumerate(((rawq, qTf), (rawk, kTf))):
                    sq0 = g2.tile([128, GW], BF16, tag="sq0")
                    sq1 = g2.tile([128, GW], BF16, tag="sq1")
                    nc.scalar.square(sq0, raw[:, 0, :])
                    nc.scalar.square(sq1, raw[:, 1, :])
                    pss = psc.tile([128, GW], F32, tag="psc")
                    nc.tensor.matmul(pss, lhsT=ones, rhs=sq0,
                                     start=True, stop=False)
                    nc.tensor.matmul(pss, lhsT=ones, rhs=sq1,
                                     start=False, stop=True)
                    # rstd = (mean_scaled + eps)^-0.5 via Ln -> Exp(-0.5 x)
                    # (ACT Rsqrt is unavailable; this pair is ~2e-5 accurate
                    # and keeps the reciprocal off the vector engine)
                    lnt = g2.tile([128, GW], F32, tag="lnt")
                    rstd = g2.tile([128, GW], F32, tag="rstd")
                    s2 = SCALING * SCALING if ti == 0 else 1.0
                    nc.scalar.activation(
                        out=lnt, in_=pss,
                        func=mybir.ActivationFunctionType.Ln,
                        scale=1.0 / (D * s2),
                        bias=(eps_q if ti == 0 else eps_k),
                    )
                    nc.scalar.activation(
                        out=rstd, in_=lnt,
                        func=mybir.ActivationFunctionType.Exp,
                        scale=-0.5,
                    )
                    # normalize in place (bf16 <- bf16 * f32)
                    for dc in range(2):
                        nc.vector.tensor_mul(raw[:, dc, :], raw[:, dc, :], rstd)
                    # rope: fin[dc] = cos*g0*raw[dc] + sin*g1*raw[1-dc]
                    for dc in range(2):
                        gb = ti * 4 + dc * 2
                        t1 = g2.tile([128, GW], BF16, tag="t1")
                        t2 = g2.tile([128, GW], BF16, tag="t2")
                        nc.vector.scalar_tensor_tensor(
                            out=t1, in0=cos_sb[:, sl],
                            scalar=gains[:, gb:gb + 1],
                            in1=raw[:, dc, :], op0=MULT, op1=MULT)
                        nc.vector.scalar_tensor_tensor(
                            out=t2, in0=sin_sb[:, sl],
                            scalar=gains[:, gb + 1:gb + 2],
                            in1=raw[:, 1 - dc, :], op0=MULT, op1=MULT)
                        nc.vector.tensor_add(fin[:, dc, sl], t1, t2)

            def attn_scores(G):
                """Scores + exp for attention group G (query rows 4G..4G+3).

                One matmul per (key block, dc), exactly as wide as the query
                rows whose sliding window contains that key block (1..4 row
                blocks, contiguous) -- no wasted score columns, and one
                LDWEIGHTS serves up to four rows.  Returns {c: (pT, qstart)}.
                """
                r0 = 4 * G
                pts = {}
                for c in range(max(0, r0 - BLK_WIN), r0 + 4):
                    qs = max(c, r0)                 # first row needing c
                    qe = min(c + BLK_WIN, r0 + 3)   # last row needing c
                    w = (qe - qs + 1) * 128
                    psW = psc.tile([128, GW], F32, tag="psc")
                    for dc in range(2):
                        nc.tensor.matmul(
                            psW[:, 0:w],
                            lhsT=kTf[:, dc, c * 128:(c + 1) * 128],
                            rhs=qTf[:, dc, qs * 128:(qe + 1) * 128],
                            start=(dc == 0), stop=(dc == 1),
                        )
                    if c >= qs and c <= qe:         # diagonal block (c == r)
                        j = (c - qs) * 128
                        nc.vector.tensor_add(
                            psW[:, j:j + 128], psW[:, j:j + 128], mask_diag)
                    if c + BLK_WIN <= qe:           # partial-window block
                        j = (c + BLK_WIN - qs) * 128
                        nc.vector.tensor_add(
                            psW[:, j:j + 128], psW[:, j:j + 128], mask_part)
                    pT = ptp.tile([128, GW], BF16, tag="pT")
                    nc.scalar.activation(
                        out=pT[:, 0:w], in_=psW[:, 0:w],
                        func=mybir.ActivationFunctionType.Exp,
                    )
                    pts[c] = (pT, qs)
                return pts

            def attn_row(r, pts, split_out=False, use_ppm=False):
                """attn-value accumulation + o_proj + out DMA for row r.

                use_ppm: after the last projection the ppm pool is idle, so
                late rows can accumulate there and overlap pav-pool rows.
                """
                cmin = max(0, r - BLK_WIN)
                if use_ppm:
                    ps_w = ppm.tile([128, GW], F32, tag="ppm")
                    ps_at = ps_w[:, 0:D + 1]
                else:
                    ps_at = pav.tile([128, D + 1], F32, tag="pav")
                for c in range(cmin, r + 1):
                    pT, qs = pts[c]
                    j = (r - qs) * 128
                    nc.tensor.matmul(
                        ps_at,
                        lhsT=pT[:, j:j + 128],
                        rhs=v_aug[:, c, :],
                        start=(c == cmin), stop=(c == r),
                    )
                rc = g2.tile([128, 1], F32, tag="rc")
                nc.vector.reciprocal(rc, ps_at[:, D:D + 1])
                a_sb = g2.tile([128, D], BF16, tag="asb")
                nc.scalar.mul(a_sb, ps_at[:, 0:D], rc)
                pt = ptr.tile([128, 2 * 128], BF16, tag="ptr")
                for dc in range(2):
                    nc.tensor.transpose(
                        pt[:, dc * 128:(dc + 1) * 128],
                        a_sb[:, dc * 128:(dc + 1) * 128], ident)
                attnT = g3.tile([128, 2 * 128], BF16, tag="attnT")
                nc.vector.tensor_copy(attnT, pt)
                # o_proj for this row block
                o_sb = g3.tile([128, H], BF16, tag="osb")
                for hc in range(H // GW):
                    ps = pop.tile([128, GW], F32, tag="pop")
                    for dc in range(2):
                        nc.tensor.matmul(
                            ps,
                            lhsT=attnT[:, dc * 128:(dc + 1) * 128],
                            rhs=wo_sb[:, dc, hc * GW:(hc + 1) * GW],
                            start=(dc == 0), stop=(dc == 1),
                        )
                    if hc % 2 == 0:
                        nc.scalar.copy(o_sb[:, hc * GW:(hc + 1) * GW], ps)
                    else:
                        nc.vector.tensor_copy(o_sb[:, hc * GW:(hc + 1) * GW], ps)
                    if split_out:
                        # kernel tail: stream each chunk out as soon as its
                        # copy lands so the final DMA is small
                        nc.sync.dma_start(
                            out=out_d[r * 128:(r + 1) * 128,
                                      hc * GW:(hc + 1) * GW],
                            in_=o_sb[:, hc * GW:(hc + 1) * GW])
                if not split_out:
                    nc.sync.dma_start(out=out_d[r * 128:(r + 1) * 128, :],
                                      in_=o_sb)

            # ---------------- pipelined main loop ----------------
            ht_tiles = {}

            def fetch_ht(g):
                ht = htpool.tile([128, KC, GW], BF16, tag="ht")
                for dq in range(5):
                    nc.sync.dma_start(
                        out=ht[:, dq * 4:(dq + 1) * 4, :],
                        in_=hT_d[:, g, dq * 4:(dq + 1) * 4, :],
                    )
                ht_tiles[g] = ht

            fetch_ht(0)
            for g in range(NG):
                ht = ht_tiles.pop(g)
                rawq = proj_qk(ht, wq_sb, "rawq")
                if g == 0:
                    load_w(wk_sb, wk_d)
                else:
                    if g + 1 < NG:
                        fetch_ht(g + 1)
                    pts = attn_scores(g - 1)
                    attn_row(4 * (g - 1) + 0, pts)
                rawk = proj_qk(ht, wk_sb, "rawk")
                if g == 0:
                    load_w(wv_sb, wv_d)
                    fetch_ht(1)
                else:
                    attn_row(4 * (g - 1) + 1, pts)
                proj_v(g, ht)
                if g == 0:
                    nc.sync.dma_start(out=mask_diag, in_=md_d[:])
                    nc.sync.dma_start(out=mask_part, in_=mp_d[:])
                    nc.sync.dma_start(out=ones, in_=ones_d[:])
                    nc.sync.dma_start(out=gains, in_=gains_d[:])
                    nc.sync.dma_start(out=cos_sb, in_=cos_d[:])
                    nc.sync.dma_start(out=sin_sb, in_=sin_d[:])
                    nc.sync.dma_start(out=wo_sb, in_=wo_d[:])
                else:
                    attn_row(4 * (g - 1) + 2, pts)
                    if g < NG - 1:
                        attn_row(4 * (g - 1) + 3, pts)
                norm_rope(g, rawq, rawk)
                if g == NG - 1:
                    # row 11 lands here so its matmuls cover the rope-3 wait
                    attn_row(4 * (g - 1) + 3, pts, use_ppm=True)
                    pts = attn_scores(NG - 1)
                    attn_row(12, pts)
                    attn_row(13, pts, use_ppm=True)
            attn_row(14, pts, split_out=True)
            attn_row(15, pts, split_out=True, use_ppm=True)

    return nc


def make_in_maps(hidden_states, cos, sin, w_qkv, w_o, q_norm_w, k_norm_w):
    """Host-side sharding / relayout: one input map per core."""
    f32 = np.float32
    hT = _chunk_part(np.ascontiguousarray(hidden_states.T).astype(f32))
    # regroup to [128, 4 seq-groups, KC, 512] so each 512-col group loads
    # with a few large contiguous DMAs
    hT = _bf16(np.ascontiguousarray(
        hT.reshape(128, KC, NG, GW).transpose(0, 2, 1, 3)))

    # cos/sin have the freqs duplicated along the last dim -> keep one half,
    # transposed to [128, S]
    cosh = _bf16(np.ascontiguousarray(np.asarray(cos, dtype=f32).T[:128]))
    sinh = _bf16(np.ascontiguousarray(np.asarray(sin, dtype=f32).T[:128]))

    def gain_cols(w):
        w1 = 1.0 + np.asarray(w, dtype=f32)
        lo, hi = w1[:128], w1[128:]
        # [t1 dc0, t2 dc0, t1 dc1, t2 dc1]
        return [lo, -hi, hi, lo]

    gains = np.stack(gain_cols(q_norm_w) + gain_cols(k_norm_w),
                     axis=1).astype(f32)  # [128, 8]

    jj = np.arange(128)[:, None]  # key index within block (partition)
    ii = np.arange(128)[None, :]  # query index within block (free)
    mask_diag = np.where(jj <= ii, 0.0, MASK_VAL).astype(f32)
    mask_part = np.where(jj >= ii + 1, 0.0, MASK_VAL).astype(f32)
    ones_h = _bf16(np.ones((128, 128), f32))

    in_maps = []
    for h in range(N_CORES):
        g = h // (NH // NKV)
        wq = _bf16(_chunk_part(np.ascontiguousarray(
            w_qkv[:, h * D:(h + 1) * D]).astype(f32)))
        wk = _bf16(_chunk_part(np.ascontiguousarray(
            w_qkv[:, NH * D + g * D: NH * D + (g + 1) * D]).astype(f32)))
        wv = _bf16(_chunk_part(np.ascontiguousarray(
            w_qkv[:, (NH + NKV) * D + g * D: (NH + NKV) * D + (g + 1) * D]
        ).astype(f32)))
        wo = _bf16(_chunk_part(np.ascontiguousarray(
            w_o[h * D:(h + 1) * D, :]).astype(f32)))
        in_maps.append({
            "hT": hT, "wq": wq, "wk": wk, "wv": wv,
            "cosh": cosh, "sinh": sinh, "gains": gains,
            "wo": wo, "mask_diag": mask_diag, "mask_part": mask_part,
            "ones": ones_h,
        })
    return in_maps


_NC_CACHE = None


def _get_nc():
    global _NC_CACHE
    if _NC_CACHE is None:
        _NC_CACHE = split_multiwaits(build_nc())
    return _NC_CACHE


def run(inputs, trace=False, **kw):
    """Returns (full_output, BassKernelResults)."""
    nc = _get_nc()
    in_maps = make_in_maps(**inputs)
    res = run_bass_kernel_spmd(
        nc, in_maps, core_ids=list(range(N_CORES)), trace=trace, **kw
    )
    parts = [res.results[i]["out"].astype(np.float32) for i in range(N_CORES)]
    out = np.sum(np.stack(parts, axis=0), axis=0, dtype=np.float32)
    return out, res


def kernel(**inputs) -> np.ndarray:
    out, _ = run(inputs, trace=False)
    return out
